# revision 29
# baseline (speedup 1.0000x reference)
"""Trainium2 Bass kernel: causal attention block with query-axis softmax.

Reference math (per batch element b):
    Q = X @ Wq + bq ; K = X @ Wk + bk ; V = X @ Wv + bv          # [T, D]
    logits[i, j] = Q[i] . K[j],  logits[i, j] = -inf where j > i
    probs = softmax(logits, axis=i) / sqrt(1024)                 # QUERY axis
    out = X + probs @ V

Distribution: pure data-parallel — B=8 batch elements, one per NeuronCore,
weights replicated, no collectives.

Per-core implementation notes (zero-bias fast path):
  * Works in "transposed logit" space LT[j, i] = logits[i, j], so the
    axis-i softmax is a per-partition free-axis reduction.
  * logits = X (Wq Wk^T) X^T: M = Wq Wk^T is computed once on device,
    then Y^T = M^T X^T and LT = X Y^T — one projection instead of two.
  * The ENTIRE matmul pipeline runs in fp8 with DoubleRow (2 contraction
    rows per PE pass, 2x bf16 throughput): X, Wq, Wk, Wv arrive from the
    host as e4m3 (weights pre-scaled by 32 to sit in e4m3's normal
    range); M and Y^T are quantized to e4m3 on device with power-of-two
    rescales (M8 = 16*M, YT8 = Y) so every stage stays in range.
    All accumulation is fp32 PSUM.  Simulated end-to-end rel err 4.5e-3
    vs the 2e-2 gate.
  * probs@V runs from e5m2 probs/V rows (range needed for exp tails)
    into which the softmax denominator, the 1/sqrt(1024) scale and the
    1/32 weight-prescale compensation are folded.
  * Causal structure: LT row-chunk jc computes only i >= 128*jc (aligned
    into its PSUM bank); probs rows live in pair-tiles so the DoubleRow
    probs@V matmuls skip fully-masked pairs.  C rows and E columns are
    interleaved (E_k traced after C_{k+1}) so probs@V matmuls fill the
    PE while softmax of later rows runs.
  * Softmax max is computed per 512-col PSUM slice while the PE streams
    the next slice (loop order: slice outer, contraction-pair inner),
    so only a tiny 4-element combine + exp sit between a C row-chunk's
    last matmul and the release of its PSUM banks.

The with_bias=True fallback (never taken for this problem's all-zero
biases) keeps the original bf16 Q/K projection structure with PE
transposes.
"""

import sys

if "/opt/trn_rl_repo" not in sys.path:
    sys.path.insert(0, "/opt/trn_rl_repo")

import numpy as np

import concourse.bass as bass
import concourse.mybir as mybir
import concourse.tile as tile
from concourse import bacc
from concourse.bass import ts
from concourse.bass_utils import run_bass_kernel_spmd

B, T, D = 8, 2048, 1024
P = 128
DC = D // P  # 8 feature chunks
TC = T // P  # 16 token chunks
NP = TC // 2  # 8 token-chunk pairs (DoubleRow)
NS = 512  # matmul moving free-dim
SL = T // NS  # 4 slices per full row
F32 = mybir.dt.float32
BF16 = mybir.dt.bfloat16
FP8E4 = mybir.dt.float8e4  # e4m3
FP8E5 = mybir.dt.float8e5  # e5m2
NEG = -1.0e30
N_CORES = 8
W_PRESCALE = 32.0  # keeps 32*W in e4m3's normal range (|W| ~ 0.02)

NP_BF16 = mybir.dt.np(BF16)
NP_FP8E4 = mybir.dt.np(FP8E4)


def host_tri_mask() -> np.ndarray:
    """[128, 128] additive mask for the diagonal block of LT row-chunk jc:
    entry [p, c] (j = jc*128+p, i = jc*128+c) is 0 where i >= j else -1e30."""
    p = np.arange(P)[:, None]
    c = np.arange(P)[None, :]
    return np.where(c >= p, 0.0, NEG).astype(np.float32)


def build_nc_fast():
    """Zero-bias fast path: all matmuls fp8 DoubleRow."""
    nc = bacc.Bacc("TRN2", target_bir_lowering=False, debug=False)

    xb_d = nc.declare_dram_parameter("x_bf16", [T, D], BF16, isOutput=False)
    tri_d = nc.declare_dram_parameter("tri_mask", [P, P], F32, isOutput=False)
    # output in bf16 (host casts back to f32): halves store traffic; the
    # residual X is also read back in bf16 (rel err 5.0e-3 vs 2e-2 gate)
    out_d = nc.declare_dram_parameter("out", [T, D], BF16, isOutput=True)
    xt8_d = nc.declare_dram_parameter("xt_fp8", [D, T], FP8E4, isOutput=False)
    wqt8_d = nc.declare_dram_parameter("wqt_fp8", [D, D], FP8E4, isOutput=False)
    wkt8_d = nc.declare_dram_parameter("wkt_fp8", [D, D], FP8E4, isOutput=False)
    wv8_d = nc.declare_dram_parameter("wv_fp8", [D, D], FP8E4, isOutput=False)

    with tile.TileContext(nc) as tc:
        with (
            tc.tile_pool(name="persist", bufs=1) as persist,
            tc.tile_pool(name="fstage", bufs=4) as fstage,
            tc.tile_pool(name="stats", bufs=4) as stats,
            tc.tile_pool(name="psum", bufs=2, space="PSUM") as psum,
        ):
            # ---- constants ----
            trimask = persist.tile([P, P], F32, tag="trimask", name="trimask")

            # ---- persistent activations / weights (all fp8) ----
            WQT8 = persist.tile([P, DC, D], FP8E4, tag="WQT8", name="WQT8")
            WKT8 = persist.tile([P, DC, D], FP8E4, tag="WKT8", name="WKT8")
            XT8 = persist.tile([P, DC, T], FP8E4, tag="XT8", name="XT8")
            Wv8 = persist.tile([P, DC, D], FP8E4, tag="Wv8", name="Wv8")
            M8 = persist.tile([P, DC, D], FP8E4, tag="M8", name="M8")
            YT8 = persist.tile([P, DC, T], FP8E4, tag="YT8", name="YT8")
            V = persist.tile([P, TC, D], FP8E5, tag="V", name="V")  # V [j, v]
            # probs^T rows in pair-tiles for DoubleRow: pair p holds rows
            # jc=2p (at [:, 0, 0:]) and jc=2p+1 (at [:, 1, 128:]), both
            # covering i in [256*p, T).
            PT = [
                persist.tile(
                    [P, 2, T - 2 * P * p], FP8E5, tag=f"PT{p}", name=f"PT{p}"
                )
                for p in range(NP)
            ]
            # row 2p+1's first 128 columns are never written by exp but are
            # read by the pair matmuls -> must be zero.
            for p in range(NP):
                nc.gpsimd.memset(PT[p][:, 1, 0:P], 0.0)

            # DMA order = consumption order: M needs WQT8/WKT8 first.
            for kc in range(DC):
                nc.sync.dma_start(out=WQT8[:, kc, :], in_=wqt8_d[ts(kc, P), :])
                nc.sync.dma_start(out=WKT8[:, kc, :], in_=wkt8_d[ts(kc, P), :])
            nc.sync.dma_start(out=trimask, in_=tri_d[:, :])
            for dc in range(DC):
                nc.sync.dma_start(out=XT8[:, dc, :], in_=xt8_d[ts(dc, P), :])
            for dc in range(DC):
                nc.sync.dma_start(out=Wv8[:, dc, :], in_=wv8_d[ts(dc, P), :])
            # bf16 copies so the causal mask can be ADDED BY THE PE (an
            # identity x trimask matmul accumulated into the logits) instead
            # of a vector-engine tensor_add in the softmax critical path.
            # Declared here; the DVE copy is traced right before the rounds
            # so it never sits in front of the M/YT quantize copies.
            from concourse.masks import make_identity

            ident_b = persist.tile([P, P], BF16, tag="ident_b", name="ident_b")
            tri_b = persist.tile([P, P], BF16, tag="tri_b", name="tri_b")

            # All PSUM allocations are uniform [P, 1024] (2 banks) with a
            # 4-deep ring, so a phase's PSUM is reclaimed 4 allocations
            # later — the softmax/exp chains never block the next row-
            # chunk's matmuls.
            def psum_acc():
                return psum.tile([P, D], F32, tag="acc", bufs=4, name="acc")



            def q_copy(dst, src, scale, idx=0):
                # PSUM f32 -> SBUF fp8 quantize-with-rescale, alternating
                # between the vector and scalar engines so neither FIFO
                # becomes the drain bottleneck.
                if idx % 2 == 0:
                    nc.vector.tensor_scalar(
                        out=dst,
                        in0=src,
                        scalar1=scale,
                        scalar2=None,
                        op0=mybir.AluOpType.mult,
                    )
                else:
                    nc.scalar.mul(dst, src, scale)

            # ---- V = X Wv (DoubleRow fp8): acc[j, v] = 32 (X Wv)[j, v] ----
            # The 1/(32*32) of (weight prescale * 1/sqrt(1024)) is folded
            # into the PSUM->SBUF quantize copy.
            def phase_v(jc):
                acc = psum_acc()
                for dp in range(DC // 2):
                    for vs in range(2):
                        nc.tensor.matmul(
                            acc[:, ts(vs, NS)],
                            lhsT=XT8[:, 2 * dp : 2 * dp + 2, ts(jc, P)],
                            rhs=Wv8[:, 2 * dp : 2 * dp + 2, ts(vs, NS)],
                            start=(dp == 0),
                            stop=(dp == DC // 2 - 1),
                            perf_mode=mybir.MatmulPerfMode.DoubleRow,
                        )
                q_copy(V[:, jc, :], acc[:, :], 1.0 / (32.0 * W_PRESCALE), idx=jc)

            # ---- M = Wq Wk^T (DoubleRow fp8) ----
            # acc[a, b] = sum_k (32Wq)[a, k] (32Wk)[b, k] = 1024 M[a, b]
            # kp-outer over 4 concurrent PSUM groups: the kp=0 matmuls only
            # need weight chunks 0-1, so the PE starts streaming while the
            # remaining weight DMAs are still in flight.
            for half in range(2):
                haccs = [psum_acc() for _ in range(4)]
                for kp in range(DC // 2):
                    for ai in range(4):
                        a = 4 * half + ai
                        for bs in range(2):
                            nc.tensor.matmul(
                                haccs[ai][:, ts(bs, NS)],
                                lhsT=WQT8[:, 2 * kp : 2 * kp + 2, ts(a, P)],
                                rhs=WKT8[:, 2 * kp : 2 * kp + 2, ts(bs, NS)],
                                start=(kp == 0),
                                stop=(kp == DC // 2 - 1),
                                perf_mode=mybir.MatmulPerfMode.DoubleRow,
                            )
                for ai in range(4):
                    a = 4 * half + ai
                    # M8 = acc / 64 = 16 M  (|16M| < ~1.4, e4m3-friendly)
                    q_copy(M8[:, a, :], haccs[ai][:, :], 1.0 / 64.0, idx=a)

            # ---- Y^T = M^T X^T (DoubleRow fp8), in 1024-col halves ----
            # acc[e, i] = sum_d (16M)[d, e] X[i, d] = 16 Y[i, e]
            for m in range(DC):
                for h in range(2):
                    acc = psum_acc()
                    for dp in range(DC // 2):
                        for s in range(2):
                            nc.tensor.matmul(
                                acc[:, ts(s, NS)],
                                lhsT=M8[:, 2 * dp : 2 * dp + 2, ts(m, P)],
                                rhs=XT8[
                                    :, 2 * dp : 2 * dp + 2, ts(2 * h + s, NS)
                                ],
                                start=(dp == 0),
                                stop=(dp == DC // 2 - 1),
                                perf_mode=mybir.MatmulPerfMode.DoubleRow,
                            )
                    # YT8 = acc / 16 = Y  (|Y| < ~3)
                    q_copy(YT8[:, m, ts(h, D)], acc[:, :], 1.0 / 16.0, idx=2 * m + h)
                phase_v(NP + m)  # V rows 8..15 ride the YT phase

            for jc in range(NP):
                phase_v(jc)
            make_identity(nc, ident_b)
            nc.vector.tensor_copy(out=tri_b, in_=trimask)

            # ====== phases C+D+E interleaved ======
            # C_jc: LT row-chunk jc (i >= 128*jc, PSUM-bank aligned) + softmax
            # E_ic: read[ic] = probs @ V + residual + store
            # Trace order C0, C1, E0, C2, E1, ..., C15, E14, E15 so E matmuls
            # fill the PE while softmax of later C rows runs.
            def phase_c(jc):
                g, r = jc // 4, jc % 4
                off = NS * g  # global acc column 0 corresponds to i = off
                L = T - off  # valid global acc cols are [dstart, L)
                dstart = P * r  # diagonal block offset
                # segments (global_acc_lo, i_lo, i_hi), each inside one PSUM
                # bank; slice-outer / pair-inner so each slice finishes early
                # and its row-max reduction overlaps the next slice's matmuls.
                segs = [(dstart, P * jc, NS * (g + 1))]
                for s in range(g + 1, SL):
                    segs.append((s * NS - off, s * NS, (s + 1) * NS))
                nseg = len(segs)
                maxs = stats.tile([P, SL], F32, tag="maxs", bufs=4, name="maxs")
                negmax1 = stats.tile(
                    [P, 1], F32, tag="negmax", bufs=4, name="negmax"
                )
                # segs si=0,1 -> psum alloc 0 (global cols [0,1024)),
                # segs si=2,3 -> psum alloc 1 (global cols [1024,2048))
                accs = []
                for gi in range((nseg + 1) // 2):
                    acc = psum_acc()
                    accs.append(acc)
                    for si in range(2 * gi, min(2 * gi + 2, nseg)):
                        alo, ilo, ihi = segs[si]
                        lalo = alo - D * gi
                        w = ihi - ilo
                        for dp in range(DC // 2):
                            nc.tensor.matmul(
                                acc[:, lalo : lalo + w],
                                lhsT=XT8[:, 2 * dp : 2 * dp + 2, ts(jc, P)],
                                rhs=YT8[:, 2 * dp : 2 * dp + 2, ilo:ihi],
                                start=(dp == 0),
                                stop=(dp == DC // 2 - 1),
                                perf_mode=mybir.MatmulPerfMode.DoubleRow,
                            )
                        if si == 0:
                            nc.tensor.matmul(
                                acc[:, dstart : dstart + P],
                                lhsT=ident_b,
                                rhs=tri_b[:, :],
                                start=False,
                                stop=True,
                                skip_group_check=True,
                            )
                        if nseg == 1:
                            nc.vector.reduce_max(
                                out=negmax1,
                                in_=acc[:, lalo : lalo + w],
                                axis=mybir.AxisListType.X,
                                negate=True,
                            )
                        else:
                            nc.vector.reduce_max(
                                out=maxs[:, si : si + 1],
                                in_=acc[:, lalo : lalo + w],
                                axis=mybir.AxisListType.X,
                                negate=False,
                            )
                if nseg == 1:
                    negmax = negmax1
                else:
                    negmax = stats.tile(
                        [P, 1], F32, tag="negmax", bufs=4, name="negmax"
                    )
                    nc.vector.reduce_max(
                        out=negmax,
                        in_=maxs[:, 0:nseg],
                        axis=mybir.AxisListType.X,
                        negate=True,
                    )
                pr, rr = jc // 2, jc % 2
                ssums = []
                cstart = dstart  # global acc col where the valid region starts
                for gi, acc in enumerate(accs):
                    lo = cstart - D * gi
                    hi = min(D, L - D * gi)
                    pt_lo = P * rr + (cstart - dstart)
                    ssum_g = stats.tile(
                        [P, 1], F32, tag="ssum", bufs=8, name="ssum"
                    )
                    nc.scalar.activation(
                        out=PT[pr][:, rr, pt_lo : pt_lo + (hi - lo)],
                        in_=acc[:, lo:hi],
                        func=mybir.ActivationFunctionType.Exp,
                        bias=negmax,
                        scale=1.0,
                        accum_out=ssum_g,
                    )
                    ssums.append(ssum_g)
                    cstart += hi - lo
                if len(ssums) == 2:
                    ssum = stats.tile(
                        [P, 1], F32, tag="ssum", bufs=8, name="ssum"
                    )
                    nc.vector.tensor_add(
                        out=ssum, in0=ssums[0], in1=ssums[1]
                    )
                else:
                    ssum = ssums[0]
                c_ssum[jc] = ssum

            c_ssum = {}

            def phase_c_post(jc):
                # reciprocal + fold of the softmax denominator into V row jc.
                # Traced one round late so exp of the next row-chunk is never
                # queued behind it.  Early rounds: scalar engine (DVE is busy
                # with 3-4 seg-maxes and V copies); late rounds: vector
                # engine (so the ACT FIFO is pure exp, whose latency gates
                # the E phases' PSUM ring).
                rv = stats.tile([P, 1], F32, tag="rv", bufs=4, name="rv")
                nc.vector.reciprocal(out=rv, in_=c_ssum[jc])
                # split across ACT and DVE: half the latency, and each E
                # matmul (vs=0/1) depends only on its own half of V row jc
                nc.scalar.mul(V[:, jc, 0:NS], V[:, jc, 0:NS], rv)
                nc.vector.tensor_scalar(
                    out=V[:, jc, NS:D],
                    in0=V[:, jc, NS:D],
                    scalar1=rv,
                    scalar2=None,
                    op0=mybir.AluOpType.mult,
                )

            def phase_e(ic):
                acc = psum_acc()
                np_ic = ic // 2 + 1  # pairs 0..ic//2
                for p in range(np_ic):
                    blk = PT[p][
                        :, :, ic * P - 2 * P * p : (ic + 1) * P - 2 * P * p
                    ]
                    for vs in range(2):
                        nc.tensor.matmul(
                            acc[:, ts(vs, NS)],
                            lhsT=blk,
                            rhs=V[:, 2 * p : 2 * p + 2, ts(vs, NS)],
                            start=(p == 0),
                            stop=(p == np_ic - 1),
                            perf_mode=mybir.MatmulPerfMode.DoubleRow,
                        )
                xf = fstage.tile([P, D], BF16, tag="xf", bufs=8, name="xf")
                nc.sync.dma_start(out=xf, in_=xb_d[ts(ic, P), :])
                ot = fstage.tile([P, D], BF16, tag="ot", bufs=4, name="ot")
                nc.vector.tensor_add(out=ot, in0=acc[:, :], in1=xf)
                nc.sync.dma_start(out=out_d[ts(ic, P), :], in_=ot)

            # lag-3 interleave: E_k needs V row k+1 scaled (end of C_{k+1}'s
            # softmax chain); tracing E_{t-3} after C_t gives that chain a
            # full extra round of slack, so E matmuls never stall on it.
            # V rows 8..15 are interleaved into the early rounds as extra PE
            # cover (early E phases are small); this also staggers the PSUM
            # ring so consecutive C phases don't serialize on each other's
            # exp.
            # Round order: early rounds trace C first (their E phases are
            # 1-2 pairs and would otherwise immediately wait on the freshest
            # V-scale); late rounds trace E first so, in the 4-deep PSUM
            # ring, an E phase ring-waits on the previous E's residual add
            # rather than on a C row-chunk's exp.
            for jc in range(TC):
                if jc >= 10:
                    phase_e(jc - 3)
                phase_c(jc)
                if jc >= 1:
                    phase_c_post(jc - 1)
                if 3 <= jc < 10:
                    phase_e(jc - 3)
            phase_c_post(TC - 1)
            phase_e(TC - 3)
            phase_e(TC - 2)
            phase_e(TC - 1)

    nc.finalize()
    return nc


def build_nc_bias():
    """Fallback with non-zero biases: original bf16 projection structure."""
    nc = bacc.Bacc("TRN2", target_bir_lowering=False, debug=False)

    x_d = nc.declare_dram_parameter("minibatch", [T, D], F32, isOutput=False)
    tri_d = nc.declare_dram_parameter("tri_mask", [P, P], F32, isOutput=False)
    out_d = nc.declare_dram_parameter("out", [T, D], F32, isOutput=True)
    wq_d = nc.declare_dram_parameter("Wq", [D, D], F32, isOutput=False)
    bq_d = nc.declare_dram_parameter("bq", [D], F32, isOutput=False)
    wk_d = nc.declare_dram_parameter("Wk", [D, D], F32, isOutput=False)
    bk_d = nc.declare_dram_parameter("bk", [D], F32, isOutput=False)
    wv_d = nc.declare_dram_parameter("Wv", [D, D], F32, isOutput=False)
    bv_d = nc.declare_dram_parameter("bv", [D], F32, isOutput=False)

    with tile.TileContext(nc) as tc:
        with (
            tc.tile_pool(name="persist", bufs=1) as persist,
            tc.tile_pool(name="wpool", bufs=8) as wpool,
            tc.tile_pool(name="fstage", bufs=4) as fstage,
            tc.tile_pool(name="stats", bufs=4) as stats,
            tc.tile_pool(name="psum", bufs=2, space="PSUM") as psum,
        ):
            trimask = persist.tile([P, P], F32, tag="trimask", name="trimask")
            nc.sync.dma_start(out=trimask, in_=tri_d[:, :])

            XT = persist.tile([P, DC, T], BF16, tag="XT", name="XT")
            XT8 = persist.tile([P, DC, T], FP8E4, tag="XT8", name="XT8")
            V = persist.tile([P, TC, D], FP8E5, tag="V", name="V")
            Wv8 = persist.tile([P, DC, D], FP8E4, tag="Wv8", name="Wv8")
            PT = [
                persist.tile(
                    [P, 2, T - 2 * P * p], FP8E5, tag=f"PT{p}", name=f"PT{p}"
                )
                for p in range(NP)
            ]
            for p in range(NP):
                nc.gpsimd.memset(PT[p][:, 1, 0:P], 0.0)

            ones = persist.tile([1, NS], BF16, tag="ones", name="ones")
            nc.vector.memset(ones, 1.0)
            b_sb = {}
            for nm, bd in (("q", bq_d), ("k", bk_d), ("v", bv_d)):
                bt = persist.tile([1, D], BF16, tag=f"bias_{nm}", name=f"bias_{nm}")
                nc.gpsimd.dma_start(out=bt, in_=bd[None, :])  # f32 -> bf16
                b_sb[nm] = bt
            nc.vector.tensor_scalar(
                out=b_sb["v"],
                in0=b_sb["v"],
                scalar1=W_PRESCALE,
                scalar2=None,
                op0=mybir.AluOpType.mult,
            )
            QT = persist.tile([P, DC, T], BF16, tag="QT", name="QT")
            KT = persist.tile([P, DC, T], BF16, tag="KT", name="KT")
            ident = persist.tile([P, P], BF16, tag="ident", name="ident")
            from concourse.masks import make_identity

            make_identity(nc, ident)

            for ic in range(TC):  # X^T via PE transpose
                xf = fstage.tile([P, D], F32, tag="f32stage", bufs=4, name="xf")
                nc.sync.dma_start(out=xf, in_=x_d[ts(ic, P), :])
                xb = fstage.tile([P, D], BF16, tag="xbf", bufs=2, name="xb")
                nc.vector.tensor_copy(out=xb, in_=xf)
                pt_ = psum.tile([P, D], BF16, tag="acc", bufs=2, name="pt_")
                for dc in range(DC):
                    nc.tensor.transpose(
                        pt_[:, ts(dc, P)], xb[:, ts(dc, P)], ident
                    )
                nc.scalar.copy(
                    out=XT[:, :, ts(ic, P)],
                    in_=pt_.rearrange("p (dc c) -> p dc c", c=P),
                )
                nc.vector.tensor_copy(
                    out=XT8[:, :, ts(ic, P)], in_=XT[:, :, ts(ic, P)]
                )

            def load_w_chunks(w_dram):
                wt = []
                for dc in range(DC):
                    w1 = wpool.tile([P, D], BF16, tag="w", bufs=8, name="w1")
                    nc.gpsimd.dma_start(out=w1, in_=w_dram[ts(dc, P), :])
                    wt.append(w1)
                return wt

            for w_dram, bkey, dst in ((wq_d, "q", QT), (wk_d, "k", KT)):
                wt = load_w_chunks(w_dram)
                for m in range(DC):
                    acc = psum.tile([P, T], F32, tag="acc", bufs=2, name="acc")
                    for dc in range(DC):
                        for s in range(SL):
                            nc.tensor.matmul(
                                acc[:, ts(s, NS)],
                                lhsT=wt[dc][:, ts(m, P)],
                                rhs=XT[:, dc, ts(s, NS)],
                                start=(dc == 0),
                                stop=False,
                            )
                    for s in range(SL):
                        nc.tensor.matmul(
                            acc[:, ts(s, NS)],
                            lhsT=b_sb[bkey][:, ts(m, P)],
                            rhs=ones[:, :],
                            start=False,
                            stop=True,
                        )
                    nc.scalar.copy(out=dst[:, m, :], in_=acc)

            wt = load_w_chunks(wv_d)
            for dc in range(DC):
                nc.vector.tensor_scalar(
                    out=Wv8[:, dc, :],
                    in0=wt[dc][:, :],
                    scalar1=W_PRESCALE,
                    scalar2=None,
                    op0=mybir.AluOpType.mult,
                )

            # V (fp8 DoubleRow) + bias
            for jc in range(TC):
                acc = psum.tile([P, T], F32, tag="acc", bufs=2, name="acc")
                for dp in range(DC // 2):
                    for vs in range(2):
                        nc.tensor.matmul(
                            acc[:, ts(vs, NS)],
                            lhsT=XT8[:, 2 * dp : 2 * dp + 2, ts(jc, P)],
                            rhs=Wv8[:, 2 * dp : 2 * dp + 2, ts(vs, NS)],
                            start=(dp == 0),
                            stop=False,
                            perf_mode=mybir.MatmulPerfMode.DoubleRow,
                        )
                for vs in range(2):
                    nc.tensor.matmul(
                        acc[:, ts(vs, NS)],
                        lhsT=ones[:, 0:P],
                        rhs=b_sb["v"][:, ts(vs, NS)],
                        start=False,
                        stop=True,
                    )
                nc.vector.tensor_copy(out=V[:, jc, :], in_=acc[:, 0:D])

            def phase_c(jc):
                g, r = jc // 4, jc % 4
                off = NS * g
                L = T - off
                dstart = P * r
                acc = psum.tile([P, T], F32, tag="acc", bufs=2, name="acc")
                for kc in range(DC):
                    nc.tensor.matmul(
                        acc[:, dstart:NS],
                        lhsT=KT[:, kc, ts(jc, P)],
                        rhs=QT[:, kc, P * jc : NS * (g + 1)],
                        start=(kc == 0),
                        stop=(kc == DC - 1),
                    )
                    for s in range(g + 1, SL):
                        nc.tensor.matmul(
                            acc[:, s * NS - off : (s + 1) * NS - off],
                            lhsT=KT[:, kc, ts(jc, P)],
                            rhs=QT[:, kc, s * NS : (s + 1) * NS],
                            start=(kc == 0),
                            stop=(kc == DC - 1),
                        )
                nc.vector.tensor_add(
                    out=acc[:, dstart : dstart + P],
                    in0=acc[:, dstart : dstart + P],
                    in1=trimask,
                )
                valid = acc[:, dstart:L]
                negmax = stats.tile(
                    [P, 1], F32, tag="negmax", bufs=4, name="negmax"
                )
                nc.vector.reduce_max(
                    out=negmax, in_=valid, axis=mybir.AxisListType.X, negate=True
                )
                ssum = stats.tile([P, 1], F32, tag="ssum", bufs=4, name="ssum")
                pr, rr = jc // 2, jc % 2
                nc.scalar.activation(
                    out=PT[pr][:, rr, P * rr : P * rr + (T - P * jc)],
                    in_=valid,
                    func=mybir.ActivationFunctionType.Exp,
                    bias=negmax,
                    scale=1.0,
                    accum_out=ssum,
                )
                rv = stats.tile([P, 1], F32, tag="rv", bufs=4, name="rv")
                nc.vector.reciprocal(out=rv, in_=ssum)
                nc.vector.tensor_scalar(
                    out=V[:, jc, :],
                    in0=V[:, jc, :],
                    scalar1=rv,
                    scalar2=1.0 / (32.0 * W_PRESCALE),
                    op0=mybir.AluOpType.mult,
                    op1=mybir.AluOpType.mult,
                )

            def phase_e(ic):
                acc = psum.tile([P, T], F32, tag="acc", bufs=2, name="acc")
                np_ic = ic // 2 + 1
                for p in range(np_ic):
                    blk = PT[p][
                        :, :, ic * P - 2 * P * p : (ic + 1) * P - 2 * P * p
                    ]
                    for vs in range(2):
                        nc.tensor.matmul(
                            acc[:, ts(vs, NS)],
                            lhsT=blk,
                            rhs=V[:, 2 * p : 2 * p + 2, ts(vs, NS)],
                            start=(p == 0),
                            stop=(p == np_ic - 1),
                            perf_mode=mybir.MatmulPerfMode.DoubleRow,
                        )
                xf = fstage.tile([P, D], F32, tag="f32stage", bufs=4, name="xf")
                nc.sync.dma_start(out=xf, in_=x_d[ts(ic, P), :])
                ot = fstage.tile([P, D], F32, tag="f32stage", bufs=4, name="ot")
                nc.vector.tensor_add(out=ot, in0=acc[:, 0:D], in1=xf)
                nc.sync.dma_start(out=out_d[ts(ic, P), :], in_=ot)

            phase_c(0)
            phase_c(1)
            for jc in range(2, TC):
                phase_c(jc)
                phase_e(jc - 2)
            phase_e(TC - 2)
            phase_e(TC - 1)

    nc.finalize()
    return nc


_NC_CACHE = {}


def get_nc(with_bias: bool = False):
    if with_bias not in _NC_CACHE:
        _NC_CACHE[with_bias] = build_nc_bias() if with_bias else build_nc_fast()
    return _NC_CACHE[with_bias]


def make_in_maps(inputs: dict) -> list[dict]:
    mb = np.ascontiguousarray(np.asarray(inputs["minibatch"], dtype=np.float32))
    assert mb.shape == (B, T, D)
    shared = {
        k: np.ascontiguousarray(np.asarray(inputs[k], dtype=np.float32))
        for k in ("Wq", "bq", "Wk", "bk", "Wv", "bv")
    }
    shared["tri_mask"] = host_tri_mask()
    # alternate layouts/dtypes of the same inputs -> no device transposes
    # or dtype-conversion passes
    shared["wqt_fp8"] = np.ascontiguousarray(shared["Wq"].T * W_PRESCALE).astype(
        NP_FP8E4
    )
    shared["wkt_fp8"] = np.ascontiguousarray(shared["Wk"].T * W_PRESCALE).astype(
        NP_FP8E4
    )
    shared["wv_fp8"] = (shared["Wv"] * W_PRESCALE).astype(NP_FP8E4)
    maps = []
    for c in range(N_CORES):
        xt = np.ascontiguousarray(mb[c].T)
        maps.append(
            {
                "minibatch": mb[c],
                "x_bf16": mb[c].astype(NP_BF16),
                "xt_fp8": xt.astype(NP_FP8E4),
                **shared,
            }
        )
    return maps


def needs_bias(inputs: dict) -> bool:
    return any(
        np.any(np.asarray(inputs[k], dtype=np.float32) != 0.0)
        for k in ("bq", "bk", "bv")
    )


def kernel(**inputs) -> np.ndarray:
    nc = get_nc(with_bias=needs_bias(inputs))
    in_maps = make_in_maps(inputs)
    res = run_bass_kernel_spmd(nc, in_maps, core_ids=list(range(N_CORES)))
    return np.stack(
        [np.asarray(res.results[c]["out"]) for c in range(N_CORES)], axis=0
    ).astype(np.float32)


if __name__ == "__main__":
    rng = np.random.default_rng(0)
    demo = {
        "minibatch": rng.standard_normal((B, T, D), dtype=np.float32),
        "Wq": rng.standard_normal((D, D), dtype=np.float32) * 0.02,
        "bq": np.zeros(D, np.float32),
        "Wk": rng.standard_normal((D, D), dtype=np.float32) * 0.02,
        "bk": np.zeros(D, np.float32),
        "Wv": rng.standard_normal((D, D), dtype=np.float32) * 0.02,
        "bv": np.zeros(D, np.float32),
    }
    out = kernel(**demo)
    print(out.shape, out.dtype)
